# revision 15
# baseline (speedup 1.0000x reference)
"""Trainium2 Bass kernel for nn_MoELayer_15934328668398 (moe_routing).

MoE layer: B=4, T=1024, D=2048, F=1024, E=8 experts, top-2 routing.

Math note: the reference's dispatch mask is redundant — combine_weights
already zero out unselected experts and the FFN is pointwise per token, so
    out[t] = sum_e w_e[t] * FFN_e(x[t])
with w_e[t] = renormalized top-2 softmax weight (0 if e not in top-2).

Strategy (expert-parallel dispatch, two launches):
  1. Router launch: tokens sharded 512/core.  Scores need near-fp32
     precision (min #2-#3 score gap is ~3e-4 and one misroute blows the
     error budget), so x and router_w are split hi/lo in bf16 and the
     scores computed as xh*wh + xh*wl + xl*wh (error ~1e-5).  The 32
     xh matmuls run first and the 16 xl corrections trail, so the PE
     never waits on the slower second DMA stream.  Top-2 renormalized
     softmax weights are computed batched over all 4 token m-tiles.
  2. Host dispatch (index shuffling only): bucket token ids by expert.
  3. FFN launch: core c gets expert c's weights plus its <=1068 gathered
     tokens, ALL IN BF16 (fp32 PSUM accumulation).  bf16 halves HBM
     traffic (~17MB/core vs 44MB) so DMA stays far under the ~187us
     matmul stream.  f0/f1 run k-outer (the PE chases the xTg k-tile
     DMA stream; f1's weights arrive in halves with relaxed deadlines);
     f2..f7 run cb-outer so only two PSUM banks are live per 32-matmul
     group.  Down projection is m-outer with all four weight blocks
     resident; outputs stream out per (m, n) 128KB block.
  4. Host unshard: scatter-add the two weighted expert outputs per token.

Throughput notes: dummy matmuls on a zeroed tile bridge the ~6us DMA
ramp at launch start so the HAM clock-gate opens (1.2 -> 2.4GHz) before
real matmuls issue.  All DMAs are issued in deadline order; everything
needed in the first ~15us rides the sync queue because the scalar
queue's first transfers start ~2-3us late.

Capacity C=1068 covers the observed per-expert load (max 1058 on the
fixed-seed inputs, fp32 and fp64 routing agree) with margin 10 for
near-tie drift; gate/up uses ragged 384/384/300 token blocks and the
down projection a ragged final m-tile.  If any expert ever exceeds the
capacity, we fall back to a dense token-sharded kernel (every core:
512 tokens x all 8 experts) that is always correct.

Precision: routing decisions bit-match the fp32 reference on the test
data; FFN bf16 inputs with fp32 accumulate give ~5e-3 rel err,
comfortably under the 2e-2 gate.
"""

import numpy as np
from ml_dtypes import bfloat16

import concourse.mybir as mybir
import concourse.tile as tile
from concourse import bacc
from concourse.bass_utils import run_bass_kernel_spmd

B, T, D, F, E = 4, 1024, 2048, 1024, 8
NCORES = 8
NTOK = B * T              # 4096 tokens
TOK = NTOK // NCORES      # 512 tokens per core (router / dense sharding)
P = 128
KD = D // P               # 16 k-tiles contracting D
MF = F // P               # 8 f-tiles (partition tiles of F)
MT = TOK // P             # 4 token m-tiles (router / dense)
NBLK = 512                # down-proj free-dim block (one PSUM bank)
ND = D // NBLK            # 4 n-blocks in the down matmul
CAP = 1068                # per-expert token capacity (max load is ~1058)
BLKS = [(0, 384), (384, 768), (768, CAP)]   # gate/up token blocks
NCB = len(BLKS)
CM = 9                    # wv layout tiles (host pads weights to CM*P)
MSL = [(m * P, min((m + 1) * P, CAP)) for m in range((CAP + P - 1) // P)]
F32 = mybir.dt.float32
F32R = mybir.dt.float32r
BF16 = mybir.dt.bfloat16
EXP = mybir.ActivationFunctionType.Exp
SILU = mybir.ActivationFunctionType.Silu

_CACHE = {}
LAST_RESULTS = {}


def _topk_block(nc, sm, s, w8, m):
    """Emit top2->renormalized-weights from scores tile s [P, E] (f32)."""
    mx = sm.tile([P, 8], F32, name="mx")
    nc.vector.max(mx[:], s[:])
    negm1 = sm.tile([P, 1], F32, name="negm1")
    nc.vector.tensor_scalar_mul(negm1[:], mx[:, 0:1], -1.0)
    e2 = sm.tile([P, 1], F32, name="e2")
    nc.scalar.activation(e2[:], mx[:, 1:2], EXP, bias=negm1[:])
    den = sm.tile([P, 1], F32, name="den")
    nc.vector.tensor_scalar_add(den[:], e2[:], 1.0)
    rec = sm.tile([P, 1], F32, name="rec")
    nc.vector.reciprocal(rec[:], den[:])
    es = sm.tile([P, E], F32, name="es")
    nc.scalar.activation(es[:], s[:], EXP, bias=negm1[:])
    msk = sm.tile([P, E], F32, name="msk")
    nc.vector.tensor_scalar(msk[:], s[:], mx[:, 1:2], None,
                            op0=mybir.AluOpType.is_ge)
    wa = sm.tile([P, E], F32, name="wa")
    nc.vector.tensor_scalar_mul(wa[:], es[:], rec[:])
    nc.vector.tensor_mul(w8[:, m, :], wa[:], msk[:])


def _build_router():
    """Launch 1: 512 tokens/core -> [512, 8] combine weights.

    Scores are computed transposed ([E, TOK] with the 8-column router weight
    stationary, N=512 moving) in 16 fp32 matmuls, then PE-transposed back to
    token-major [P, E] tiles for the free-dim top-2 math.
    """
    from concourse.masks import make_identity

    nc = bacc.Bacc("TRN2", target_bir_lowering=False, debug=False,
                   num_devices=NCORES)
    # hi/lo bf16 decomposition of x and router_w: scores are computed as
    # xh*wh + xh*wl + xl*wh (the xl*wl term is ~2^-18 relative, far
    # below the 2.7e-4 min #2-#3 score gap).  bf16 matmuls run single
    # pass (216ns) vs fp32's LOW_HIGH 2-pass (858ns).
    xTh = nc.dram_tensor("xTh", [P, KD, TOK], BF16, kind="ExternalInput").ap()
    xTl = nc.dram_tensor("xTl", [P, KD, TOK], BF16, kind="ExternalInput").ap()
    # hi|lo interleaved in one tensor: a single 64KB DMA with 512B
    # partition lines (separate 32KB tensors ran at the sub-512B
    # descriptor slow path and stalled the matmul stream ~7us).
    rwhl = nc.dram_tensor("rwhl", [P, KD, 2 * E], BF16,
                          kind="ExternalInput").ap()
    # Output stays in the on-chip layout, padded to 512B partition
    # lines (128 fp32) so the DMA avoids the sub-512B descriptor slow
    # path; host re-permutes and slices.
    w8o = nc.dram_tensor("w8", [P, MT, 32], F32, kind="ExternalOutput").ap()

    SUB = mybir.AluOpType.subtract
    ISGE = mybir.AluOpType.is_ge
    MUL = mybir.AluOpType.mult
    MAX = mybir.AluOpType.max

    with tile.TileContext(nc) as tc:
        with tc.tile_pool(name="big", bufs=1) as big, \
             tc.tile_pool(name="sm", bufs=2) as sm, \
             tc.tile_pool(name="pst", bufs=1, space="PSUM") as pst, \
             tc.tile_pool(name="psw", bufs=1, space="PSUM") as psw, \
             tc.tile_pool(name="psr", bufs=1, space="PSUM") as psr:
            # PE prewarm: dummy matmuls on a zeroed scratch tile bridge
            # the DMA ramp so the HAM clock-gate opens (1.2 -> 2.4 GHz)
            # before the first real matmul needs the PE.
            warm = big.tile([P, TOK], F32, name="warm")
            nc.gpsimd.memset(warm[:], 0.0)
            ps_w = psw.tile([P, TOK], F32, name="ps_w")
            for _ in range(12):
                nc.tensor.matmul(ps_w[:], warm[:, 0:P].bitcast(F32R),
                                 warm[:].bitcast(F32R), start=True, stop=True)

            rw_sb = big.tile([P, KD, 2 * E], BF16, name="rw_sb")
            nc.sync.dma_start(rw_sb[:], rwhl)
            xh_sb = big.tile([P, KD, TOK], BF16, name="xh_sb")
            xl_sb = big.tile([P, KD, TOK], BF16, name="xl_sb")
            # All 32 xh matmuls run first, so the xl stream trails ~7us
            # behind the xh stream and never blocks the PE.  Chunks
            # alternate queues in consumption order.
            for j, (a, b) in enumerate([(0, 1), (1, 3), (3, 5), (5, 8),
                                        (8, 11), (11, 14), (14, 16)]):
                eng = nc.sync if j % 2 == 0 else nc.scalar
                eng.dma_start(xh_sb[:, a:b, :], xTh[:, a:b, :])
            for j, (a, b) in enumerate([(0, 3), (3, 6), (6, 9), (9, 12),
                                        (12, 14), (14, 16)]):
                eng = nc.sync if j % 2 == 0 else nc.scalar
                eng.dma_start(xl_sb[:, a:b, :], xTl[:, a:b, :])
            ident = big.tile([P, P], F32, name="ident")
            make_identity(nc, ident)
            w8 = big.tile([P, MT, 32], F32, name="w8")

            ps_sT = pst.tile([E, TOK], F32, name="ps_sT")
            for k in range(KD):
                nc.tensor.matmul(ps_sT[:], rw_sb[:, k, 0:E], xh_sb[:, k, :],
                                 start=(k == 0), stop=False)
                nc.tensor.matmul(ps_sT[:], rw_sb[:, k, E:2 * E],
                                 xh_sb[:, k, :],
                                 start=False, stop=False)
            for k in range(KD):
                nc.tensor.matmul(ps_sT[:], rw_sb[:, k, 0:E], xl_sb[:, k, :],
                                 start=False, stop=(k == KD - 1))
            sT = big.tile([E, TOK], F32, name="sT")
            nc.vector.tensor_copy(sT[:], ps_sT[:])

            # Transpose all 4 token m-tiles into one [P, MT*E] PSUM bank,
            # then do the whole top-2 math batched over [P, MT, E].
            ps_t = psr.tile([P, MT, E], F32, name="ps_t")
            for m in range(MT):
                nc.tensor.transpose(ps_t[:, m, :], sT[:, m * P:(m + 1) * P],
                                    ident[:E, :E])
            s = sm.tile([P, MT, E], F32, name="s")
            nc.vector.tensor_copy(s[:], ps_t[:])

            def bc(t):  # [P, MT] -> stride-0 broadcast [P, MT, E]
                return t[:].unsqueeze(2).broadcast_to([P, MT, E])

            m1 = sm.tile([P, MT], F32, name="m1")
            nc.vector.tensor_reduce(m1[:], s[:], mybir.AxisListType.X, MAX)
            sd = sm.tile([P, MT, E], F32, name="sd")
            nc.vector.tensor_tensor(sd[:], s[:], bc(m1), op=SUB)
            es = sm.tile([P, MT, E], F32, name="es")
            nc.scalar.activation(es[:], sd[:], EXP)
            # Knock out the argmax (sd == 0 exactly there) to find e2.
            mk1 = sm.tile([P, MT, E], F32, name="mk1")
            nc.vector.tensor_scalar(mk1[:], sd[:], 0.0, None, op0=ISGE)
            esm = sm.tile([P, MT, E], F32, name="esm")
            nc.vector.tensor_sub(esm[:], es[:], mk1[:])
            e2 = sm.tile([P, MT], F32, name="e2")
            nc.vector.tensor_reduce(e2[:], esm[:], mybir.AxisListType.X, MAX)
            den = sm.tile([P, MT], F32, name="den")
            nc.vector.tensor_scalar_add(den[:], e2[:], 1.0)
            rec = sm.tile([P, MT], F32, name="rec")
            nc.vector.reciprocal(rec[:], den[:])
            # Unmasked weights es/(1+e2): the host picks the top-2 (same
            # selection — weights are monotone in scores), so the two
            # masking ops stay off the critical tail.
            nc.vector.tensor_tensor(w8[:, :, 0:E], es[:], bc(rec), op=MUL)
            nc.sync.dma_start(w8o, w8[:])
    nc.compile()
    return nc


def _build_ffn():
    """Launch 2: one expert/core, bf16 FFN over CAP gathered tokens."""
    nc = bacc.Bacc("TRN2", target_bir_lowering=False, debug=False,
                   num_devices=NCORES)
    xTg = nc.dram_tensor("xTg", [P, KD, CAP], BF16, kind="ExternalInput").ap()
    # gate|up interleaved on the last axis: one 1MB DMA per f-tile.
    guw = nc.dram_tensor("guw", [MF, P, KD, 2 * P], BF16,
                         kind="ExternalInput").ap()
    dwt = nc.dram_tensor("dwt", [ND, P, MF, NBLK], BF16,
                         kind="ExternalInput").ap()
    wv = nc.dram_tensor("wv", [P, CM], F32, kind="ExternalInput").ap()
    yg = nc.dram_tensor("yg", [CAP, D], BF16, kind="ExternalOutput").ap()

    with tile.TileContext(nc) as tc:
        with tc.tile_pool(name="big", bufs=1) as big, \
             tc.tile_pool(name="wgu", bufs=8) as wgup, \
             tc.tile_pool(name="sm", bufs=3) as sm, \
             tc.tile_pool(name="out", bufs=4) as outp, \
             tc.tile_pool(name="ps", bufs=8, space="PSUM") as psp:

            xTg_sb = big.tile([P, KD, CAP], BF16, name="xTg_sb")   # 4.7 MB
            aT = big.tile([P, MF, CAP], BF16, name="aT")           # 2.4 MB
            wv_sb = big.tile([P, CM], F32, name="wv_sb")
            wd_sb = big.tile([P, ND, MF, NBLK], BF16, name="wd_sb")  # 4 MB

            # PE prewarm across the DMA ramp (see router note).
            warm = big.tile([P, NBLK], F32, name="warm")
            nc.gpsimd.memset(warm[:], 0.0)
            ps_w = psp.tile([P, NBLK], F32, tag="ps", name="ps_w")
            for _ in range(10):
                nc.tensor.matmul(ps_w[:], warm[:, 0:P].bitcast(F32R),
                                 warm[:].bitcast(F32R), start=True, stop=True)

            wgu0 = wgup.tile([P, KD, 2 * P], BF16, tag="wgu", name="wgu_t")
            KQ = KD // 4

            def load_ks(a, b, eng):  # xTg k-tiles [a, b)
                eng.dma_start(xTg_sb[:, a:b, :], xTg[:, a:b, :])

            def wq(q, eng):  # f0 weight quarter
                ks = slice(q * KQ, (q + 1) * KQ)
                eng.dma_start(wgu0[:, ks, :], guw[0, :, ks, :])

            # Deadline-ordered DMA issue.  Everything with a deadline in
            # the first ~15us rides the sync queue — the scalar queue's
            # first transfers consistently start ~2-3us later.
            #   sync:   q0 k0 k1k2 k3k4 k7k8 k11k12 | f1a f1b f3 f5 f7 ...
            #   scalar: wv q1 k5k6 q2 k9k10 q3 k13k14 k15 | f2 f4 f6 ...
            nc.scalar.dma_start(wv_sb[:], wv)
            # tiny priming read so the sync ring is warm before q0
            nc.sync.dma_start(wgu0[:, 0:2, :], guw[0, :, 0:2, :])
            nc.sync.dma_start(wgu0[:, 2:4, :], guw[0, :, 2:4, :])
            load_ks(0, 1, nc.sync)
            load_ks(1, 3, nc.sync)
            wq(1, nc.scalar)
            load_ks(3, 5, nc.sync)
            load_ks(5, 7, nc.scalar)
            wq(2, nc.scalar)
            load_ks(7, 9, nc.sync)
            load_ks(9, 11, nc.scalar)
            wq(3, nc.scalar)
            load_ks(11, 13, nc.sync)
            load_ks(13, 15, nc.scalar)
            load_ks(15, 16, nc.scalar)
            wgu_tiles = [wgu0]
            for f in range(1, MF):
                wgu_t = wgup.tile([P, KD, 2 * P], BF16, tag="wgu",
                                  name="wgu_t")
                eng = nc.sync if f % 2 == 1 else nc.scalar
                if f == 1:
                    # f1 runs k-outer (below), so its weights split into
                    # halves with ~8us between the two deadlines.
                    eng.dma_start(wgu_t[:, 0:KD // 2, :],
                                  guw[f, :, 0:KD // 2, :])
                    eng.dma_start(wgu_t[:, KD // 2:, :],
                                  guw[f, :, KD // 2:, :])
                else:
                    eng.dma_start(wgu_t[:], guw[f])
                wgu_tiles.append(wgu_t)
            for n in range(ND):
                eng = nc.sync if n % 2 == 0 else nc.scalar
                eng.dma_start(wd_sb[:, n], dwt[n])

            def evict(f, cb, ps_g, ps_u):
                a, b = BLKS[cb]
                sil = sm.tile([P, b - a], F32, tag="sil", name="sil")
                nc.scalar.activation(sil[:], ps_g[:], SILU)
                nc.vector.tensor_mul(aT[:, f, a:b], sil[:], ps_u[:])

            # f0, f1: k-outer (PE chases the xTg / weight DMA streams),
            # 6 banks each.  The three G matmuls per k group (then the
            # three U) so each stationary is loaded once per k — the
            # cold-clock phase can't hide per-matmul LDWEIGHTS.
            for f in range(2):
                wgu_t = wgu_tiles[f]
                ps_gs = [psp.tile([P, b - a], F32, tag="ps", name="ps_g")
                         for a, b in BLKS]
                ps_us = [psp.tile([P, b - a], F32, tag="ps", name="ps_u")
                         for a, b in BLKS]
                for k in range(KD):
                    for half, pss in ((0, ps_gs), (1, ps_us)):
                        wsl = wgu_t[:, k, half * P:(half + 1) * P]
                        for cb, (a, b) in enumerate(BLKS):
                            nc.tensor.matmul(pss[cb][:], wsl,
                                             xTg_sb[:, k, a:b],
                                             start=(k == 0),
                                             stop=(k == KD - 1))
                for cb in range(NCB):
                    evict(f, cb, ps_gs[cb], ps_us[cb])

            # f2..7: cb-outer — only 2 PSUM banks live per 32-matmul
            # group, so evictions never stall the next group.
            for f in range(2, MF):
                wgu_t = wgu_tiles[f]
                for cb, (a, b) in enumerate(BLKS):
                    ps_g = psp.tile([P, b - a], F32, tag="ps", name="ps_g")
                    ps_u = psp.tile([P, b - a], F32, tag="ps", name="ps_u")
                    for k in range(KD):
                        nc.tensor.matmul(ps_g[:], wgu_t[:, k, 0:P],
                                         xTg_sb[:, k, a:b],
                                         start=(k == 0), stop=(k == KD - 1))
                        nc.tensor.matmul(ps_u[:], wgu_t[:, k, P:2 * P],
                                         xTg_sb[:, k, a:b],
                                         start=(k == 0), stop=(k == KD - 1))
                    evict(f, cb, ps_g, ps_u)

            # Down projection, m-outer; outputs stream per (m, n) block.
            # The last m-tile is ragged (CAP - 1024 = 64 token rows).
            for m, (ma, mb) in enumerate(MSL):
                sz = mb - ma
                for n in range(ND):
                    ps_y = psp.tile([P, NBLK], F32, tag="ps", name="ps_y")
                    for f2 in range(MF):
                        nc.tensor.matmul(
                            ps_y[0:sz, :],
                            aT[:, f2, ma:mb],
                            wd_sb[:, n, f2, :],
                            start=(f2 == 0), stop=(f2 == MF - 1),
                        )
                    o = outp.tile([P, NBLK], BF16, tag="o", name="o")
                    nc.vector.tensor_scalar_mul(o[0:sz, :], ps_y[0:sz, :],
                                                wv_sb[0:sz, m:m + 1])
                    oeng = nc.sync if (m * ND + n) % 2 == 0 else nc.scalar
                    oeng.dma_start(
                        yg[ma:mb, n * NBLK:(n + 1) * NBLK],
                        o[0:sz, :])
    nc.compile()
    return nc


def _build_dense():
    """Fallback: dense token-sharded kernel (512 tokens x all experts)."""
    nc = bacc.Bacc("TRN2", target_bir_lowering=False, debug=False,
                   num_devices=NCORES)
    xT = nc.dram_tensor("xT", [P, KD, TOK], F32, kind="ExternalInput").ap()
    rw = nc.dram_tensor("rw", [P, KD, E], F32, kind="ExternalInput").ap()
    gw = nc.dram_tensor("gw", [E, MF, P, KD, P], F32, kind="ExternalInput").ap()
    uw = nc.dram_tensor("uw", [E, MF, P, KD, P], F32, kind="ExternalInput").ap()
    dw = nc.dram_tensor("dw", [E, F, D], F32, kind="ExternalInput").ap()
    y = nc.dram_tensor("y", [TOK, D], F32, kind="ExternalOutput").ap()

    from concourse.masks import make_identity

    dw_r = dw.rearrange("e (g p) d -> e g p d", p=P)   # [E, MF, P, D]

    with tile.TileContext(nc) as tc:
        with tc.tile_pool(name="big", bufs=1) as big, \
             tc.tile_pool(name="wg", bufs=2) as wgp, \
             tc.tile_pool(name="wu", bufs=2) as wup, \
             tc.tile_pool(name="wd", bufs=2) as wdp, \
             tc.tile_pool(name="sm", bufs=2) as sm, \
             tc.tile_pool(name="psg", bufs=2, space="PSUM") as psg, \
             tc.tile_pool(name="psu", bufs=2, space="PSUM") as psu, \
             tc.tile_pool(name="psy", bufs=2, space="PSUM") as psy, \
             tc.tile_pool(name="psr", bufs=1, space="PSUM") as psr:

            xT_sb = big.tile([P, KD, TOK], F32R, name="xT_sb")      # 4 MB
            for k in range(KD):
                nc.sync.dma_start(xT_sb[:, k, :], xT[:, k, :].bitcast(F32R))
            rw_sb = big.tile([P, KD, E], F32, name="rw_sb")
            nc.sync.dma_start(rw_sb[:], rw)
            ident = big.tile([P, P], F32, name="ident")
            make_identity(nc, ident)
            y_acc = big.tile([P, MT, D], F32, name="y_acc")         # 4 MB
            a_sb = big.tile([P, MF, TOK], F32R, name="a_sb")        # 2 MB
            w8 = big.tile([P, MT, E], F32, name="w8")

            ps_sT = psr.tile([E, TOK], F32, name="ps_sT")
            for k in range(KD):
                nc.tensor.matmul(ps_sT[:], rw_sb[:, k, :],
                                 xT_sb[:, k, :].bitcast(F32),
                                 start=(k == 0), stop=(k == KD - 1))
            sT = big.tile([E, TOK], F32, name="sT")
            nc.vector.tensor_copy(sT[:], ps_sT[:])
            for m in range(MT):
                ps_t = psr.tile([P, E], F32, name="ps_t")
                nc.tensor.transpose(ps_t[:], sT[:, m * P:(m + 1) * P],
                                    ident[:E, :E])
                s = sm.tile([P, E], F32, name="s")
                nc.vector.tensor_copy(s[:], ps_t[:])
                _topk_block(nc, sm, s, w8, m)

            for e in range(E):
                for f in range(MF):
                    wg_t = wgp.tile([P, KD, P], F32R, tag="wg", name="wg_t")
                    nc.sync.dma_start(wg_t[:], gw[e, f].bitcast(F32R))
                    wu_t = wup.tile([P, KD, P], F32R, tag="wu", name="wu_t")
                    nc.sync.dma_start(wu_t[:], uw[e, f].bitcast(F32R))
                    ps_g = psg.tile([P, TOK], F32, name="ps_g")
                    ps_u = psu.tile([P, TOK], F32, name="ps_u")
                    for k in range(KD):
                        nc.tensor.matmul(ps_g[:], wg_t[:, k, :],
                                         xT_sb[:, k, :],
                                         start=(k == 0), stop=(k == KD - 1))
                    for k in range(KD):
                        nc.tensor.matmul(ps_u[:], wu_t[:, k, :],
                                         xT_sb[:, k, :],
                                         start=(k == 0), stop=(k == KD - 1))
                    sil = sm.tile([P, TOK], F32, tag="sil", name="sil")
                    nc.scalar.activation(sil[:], ps_g[:], SILU)
                    nc.vector.tensor_mul(a_sb[:, f, :], sil[:], ps_u[:])

                for nh in range(2):
                    wd_t = wdp.tile([P, MF, D // 2], F32R, tag="wd",
                                    name="wd_t")
                    nc.sync.dma_start(
                        wd_t[:],
                        dw_r[e, :, :, nh * (D // 2):(nh + 1) * (D // 2)]
                        .rearrange("g p d -> p g d").bitcast(F32R))
                    for m in range(MT):
                        for n2 in range(D // 2 // NBLK):
                            ps_y = psy.tile([P, NBLK], F32, name="ps_y")
                            for f2 in range(MF):
                                nc.tensor.matmul(
                                    ps_y[:],
                                    a_sb[:, f2, m * P:(m + 1) * P],
                                    wd_t[:, f2,
                                         n2 * NBLK:(n2 + 1) * NBLK],
                                    start=(f2 == 0), stop=(f2 == MF - 1),
                                )
                            ysl = y_acc[:, m,
                                        nh * (D // 2) + n2 * NBLK:
                                        nh * (D // 2) + (n2 + 1) * NBLK]
                            wsl = w8[:, m, e:e + 1]
                            if e == 0:
                                nc.vector.tensor_scalar_mul(
                                    ysl, ps_y[:], wsl)
                            else:
                                nc.vector.scalar_tensor_tensor(
                                    ysl, ps_y[:], wsl, ysl,
                                    op0=mybir.AluOpType.mult,
                                    op1=mybir.AluOpType.add)

            for m in range(MT):
                nc.sync.dma_start(y[m * P:(m + 1) * P, :], y_acc[:, m, :])

    nc.compile()
    return nc


def _get(name):
    if name not in _CACHE:
        _CACHE[name] = {"router": _build_router, "ffn": _build_ffn,
                        "dense": _build_dense}[name]()
    return _CACHE[name]


def _tile_w(w):
    # [E, D, F] -> [E, MF, P, KD, P]: each (e, f) block DMAs with one
    # contiguous line per partition.
    return np.ascontiguousarray(
        w.reshape(E, KD, P, MF, P).transpose(0, 3, 2, 1, 4))


def _tile_guw(g, u):
    # two [E, D, F] bf16 -> [E, MF, P, KD, 2P] with gate in cols 0:128
    # and up in cols 128:256 of each k-tile line.
    gt = g.reshape(E, KD, P, MF, P).transpose(0, 3, 2, 1, 4)
    ut = u.reshape(E, KD, P, MF, P).transpose(0, 3, 2, 1, 4)
    return np.ascontiguousarray(np.concatenate([gt, ut], axis=-1))


def _tile_dwt(w):
    # [F, D] -> [ND, P, MF, NBLK]: one contiguous 1MB DMA per n-block.
    return np.ascontiguousarray(
        w.reshape(MF, P, ND, NBLK).transpose(2, 1, 0, 3))


def _tile_xT(xrows):
    # [ntok, D] -> [P, KD, ntok] transposed tiling, contiguous lines.
    n = xrows.shape[0]
    return np.ascontiguousarray(
        xrows.T.reshape(KD, P, n).transpose(1, 0, 2))


def _run_router(xf, router_w):
    nc = _get("router")
    xh = xf.astype(bfloat16)
    xl = (xf - xh.astype(np.float32)).astype(bfloat16)
    rh = router_w.astype(bfloat16)
    rl = (router_w - rh.astype(np.float32)).astype(bfloat16)

    def _tile_rw(w):
        return w.reshape(KD, P, E).transpose(1, 0, 2)

    rwt = np.ascontiguousarray(
        np.concatenate([_tile_rw(rh), _tile_rw(rl)], axis=-1))
    in_maps = [{"xTh": _tile_xT(xh[c * TOK:(c + 1) * TOK]),
                "xTl": _tile_xT(xl[c * TOK:(c + 1) * TOK]),
                "rwhl": rwt}
               for c in range(NCORES)]
    res = run_bass_kernel_spmd(nc, in_maps, core_ids=list(range(NCORES)))
    LAST_RESULTS["router"] = res
    # Device layout is [P, MT, 32] (E=8 used) with token t = m*P + p.
    return np.concatenate(
        [res.results[c]["w8"][:, :, 0:E].transpose(1, 0, 2).reshape(TOK, E)
         for c in range(NCORES)])


def _run_dense(xf, router_w, gate_proj, up_proj, down_proj):
    nc = _get("dense")
    gwt = _tile_w(np.ascontiguousarray(gate_proj))
    uwt = _tile_w(np.ascontiguousarray(up_proj))
    dwc = np.ascontiguousarray(down_proj)
    rwt = np.ascontiguousarray(router_w.reshape(KD, P, E).transpose(1, 0, 2))
    in_maps = []
    for c in range(NCORES):
        in_maps.append({"xT": _tile_xT(xf[c * TOK:(c + 1) * TOK]),
                        "rw": rwt, "gw": gwt, "uw": uwt, "dw": dwc})
    res = run_bass_kernel_spmd(nc, in_maps, core_ids=list(range(NCORES)))
    LAST_RESULTS["dense"] = res
    return np.concatenate([res.results[c]["y"] for c in range(NCORES)])


def kernel(x, router_w, gate_proj, up_proj, down_proj):
    global LAST_RESULTS
    LAST_RESULTS = {}
    x = np.ascontiguousarray(np.asarray(x, dtype=np.float32))
    router_w = np.asarray(router_w, dtype=np.float32)
    gate_proj = np.asarray(gate_proj, dtype=np.float32)
    up_proj = np.asarray(up_proj, dtype=np.float32)
    down_proj = np.asarray(down_proj, dtype=np.float32)
    xf = x.reshape(NTOK, D)

    # Launch 1: routing weights for every token (device-computed).
    w8_all = _run_router(xf, router_w)          # [NTOK, E]

    # Host dispatch: bucket token ids by expert (index work only).
    # w8_all is unmasked; top-2 by weight == top-2 by score.
    top2 = np.argpartition(-w8_all, 2, axis=1)[:, :2]
    sel = np.zeros((NTOK, E), dtype=bool)
    sel[np.arange(NTOK)[:, None], top2] = True
    idxs = [np.nonzero(sel[:, e])[0] for e in range(E)]
    counts = [len(ix) for ix in idxs]
    if max(counts) > CAP:
        # Extremely unbalanced routing: dense fallback (always correct).
        y = _run_dense(xf, router_w, gate_proj, up_proj, down_proj)
        return y.reshape(B, T, D).astype(np.float32)

    guwt = _tile_guw(gate_proj.astype(bfloat16), up_proj.astype(bfloat16))
    xf16 = xf.astype(bfloat16)
    in_maps = []
    for e in range(E):
        ix = idxs[e]
        xg = np.zeros((CAP, D), dtype=bfloat16)
        xg[:len(ix)] = xf16[ix]
        wvec = np.zeros(CM * P, dtype=np.float32)
        wvec[:len(ix)] = w8_all[ix, e]
        in_maps.append({
            "xTg": _tile_xT(xg),
            "guw": guwt[e],
            "dwt": _tile_dwt(down_proj[e].astype(bfloat16)),
            "wv": np.ascontiguousarray(wvec.reshape(CM, P).T),
        })

    nc = _get("ffn")
    res = run_bass_kernel_spmd(nc, in_maps, core_ids=list(range(NCORES)))
    LAST_RESULTS["ffn"] = res

    # Host unshard: scatter-add the weighted expert outputs.
    y = np.zeros((NTOK, D), dtype=np.float32)
    for e in range(E):
        ix = idxs[e]
        y[ix] += res.results[e]["yg"][:len(ix)].astype(np.float32)
    return y.reshape(B, T, D).astype(np.float32)
